# revision 10
# baseline (speedup 1.0000x reference)
"""Trainium2 Bass kernel for nn_DifferentiableStack (B=1024, L=1024, D=128, STACK=32).

Key simplification: in the reference, the push/pop gates broadcast over all
stack slots identically and the initial stack is zero, so every slot holds the
same vector. The output top-of-stack is just the scalar linear recurrence
    h_t = h_{t-1} * (1 - o_t) + x_t * p_t,      out = h_{L-1}
which unrolls to a weighted reduction over time:
    out[b,:] = sum_t x[b,t,:] * w[b,t],   w[b,t] = p[b,t] * prod_{s>t}(1 - o[b,s]).

Sharding: pure data parallel, batch dim 1024 -> 8 cores x 128 rows.

Per-core program (Tile framework):
  Phase A (~16us, hidden under Phase B's DMA): load gates [128b, L]; compute
    a = 1-o; suffix products via log2(L) shifted elementwise multiplies on a
    [128, 2L] ones-padded buffer; w = p * (shifted suffix); 8 TensorE
    transposes -> w_T [128t, tb, 128b].
  Phase B (memory-bound, streams the 64MB x shard): x tiles DMA'd as
    [128t, 16b, 128d] (512B contiguous runs); per (b, t-block) one small
    TensorE matmul  psum[d, b] = x_tile[:,b,:].T @ w_col; per t-block one
    DVE add of the psum [128d, 128b] into an SBUF accumulator.
  Output: acc [128d, 128b] -> DRAM; host transposes each core's 128x128.
"""

import numpy as np

B_TOTAL, L, D = 1024, 1024, 128
N_CORES = 8
B_LOC = B_TOTAL // N_CORES  # 128

_NC_CACHE = {}

# build configuration (overridable for experiments)
CONFIG = {
    "BC": 16,
    "x_bufs": 6,
    # NOTE: alternating HWDGE rings ("sync", "scalar") intermittently wedges
    # the device (NRT_EXEC_UNIT_UNRECOVERABLE); single-ring sync is stable.
    "dma_engines": ("sync",),
    "gpsimd_identity": True,
}


def _build_nc(L=1024, BC=16, x_bufs=6, loop_k=None, dma_engines=("sync", "scalar"),
              gpsimd_identity=True):
    import concourse.bacc as bacc
    import concourse.mybir as mybir
    import concourse.tile as tile
    from concourse import masks

    F32 = mybir.dt.float32
    B, Dd = 128, 128
    TB = L // 128
    STEPS = (L - 1).bit_length()
    assert 1 << STEPS == L

    nc = bacc.Bacc("TRN2", target_bir_lowering=False, debug=False, num_devices=8)
    x_dram = nc.dram_tensor("x", [B, L, Dd], F32, kind="ExternalInput")
    pg_dram = nc.dram_tensor("pg", [B, L], F32, kind="ExternalInput")
    og_dram = nc.dram_tensor("og", [B, L], F32, kind="ExternalInput")
    out_dram = nc.dram_tensor("out", [Dd, B], F32, kind="ExternalOutput")
    ident_dram = None
    if not gpsimd_identity:
        ident_dram = nc.dram_tensor("ident", [128, 128], F32, kind="ExternalInput")

    with tile.TileContext(nc) as tc:
        with (
            tc.tile_pool(name="const", bufs=1) as cpool,
            tc.tile_pool(name="gates", bufs=2) as gpool,
            tc.tile_pool(name="xtiles", bufs=x_bufs) as xpool,
            tc.tile_pool(name="pst", bufs=2, space="PSUM") as ppool,
            tc.tile_pool(name="psmm", bufs=2, space="PSUM") as mmpool,
            tc.tile_pool(name="outp", bufs=2) as opool,
        ):
            ident = cpool.tile([128, 128], F32)
            if gpsimd_identity:
                masks.make_identity(nc, ident[:])
            else:
                # avoid gpsimd entirely: identity comes from host as input
                nc.sync.dma_start(ident[:], ident_dram[:])

            def body(_iv=None):
                og_sb = gpool.tile([B, L], F32, tag="og")
                pg_sb = gpool.tile([B, L], F32, tag="pg")
                nc.sync.dma_start(og_sb[:], og_dram[:])
                nc.sync.dma_start(pg_sb[:], pg_dram[:])

                A0 = gpool.tile([B, 2 * L], F32, tag="A0")
                A1 = gpool.tile([B, 2 * L], F32, tag="A1")
                nc.vector.memset(A0[:, L : 2 * L], 1.0)
                nc.vector.memset(A1[:, L : 2 * L], 1.0)
                nc.vector.tensor_scalar(
                    A0[:, 0:L], og_sb[:], -1.0, 1.0,
                    op0=mybir.AluOpType.mult, op1=mybir.AluOpType.add,
                )
                cur, nxt = A0, A1
                for k in range(STEPS):
                    s = 1 << k
                    nc.vector.tensor_tensor(
                        nxt[:, 0:L], cur[:, 0:L], cur[:, s : s + L],
                        op=mybir.AluOpType.mult,
                    )
                    cur, nxt = nxt, cur
                w_bt = gpool.tile([B, L], F32, tag="wbt")
                nc.vector.tensor_tensor(
                    w_bt[:], pg_sb[:], cur[:, 1 : L + 1], op=mybir.AluOpType.mult
                )
                w_T = gpool.tile([128, TB, B], F32, tag="wT")
                for tb in range(TB):
                    pt = ppool.tile([128, 128], F32, tag="pt")
                    nc.tensor.transpose(
                        pt[:], w_bt[:, tb * 128 : (tb + 1) * 128], ident[:]
                    )
                    nc.vector.tensor_copy(w_T[:, tb, :], pt[:])

                acc = opool.tile([Dd, B], F32, tag="acc")
                n_chunks = B // BC
                for tb in range(TB):
                    mm = mmpool.tile([Dd, B], F32, tag="mm")
                    for ci in range(n_chunks):
                        xt = xpool.tile([128, BC, Dd], F32, tag="xt")
                        src = x_dram[
                            ci * BC : (ci + 1) * BC, tb * 128 : (tb + 1) * 128, :
                        ].transpose([1, 0, 2])
                        eng = getattr(
                            nc, dma_engines[(tb * n_chunks + ci) % len(dma_engines)]
                        )
                        eng.dma_start(xt[:], src)
                        for j in range(BC):
                            b = ci * BC + j
                            nc.tensor.matmul(
                                mm[:, b : b + 1],
                                xt[:, j, :],
                                w_T[:, tb, b : b + 1],
                            )
                    if tb == 0:
                        nc.vector.tensor_copy(acc[:], mm[:])
                    else:
                        nc.vector.tensor_tensor(
                            acc[:], acc[:], mm[:], op=mybir.AluOpType.add
                        )
                nc.sync.dma_start(out_dram[:], acc[:])

            if loop_k is None:
                body()
            else:
                with tc.For_i(0, loop_k, 1) as iv:
                    body(iv)

    nc.compile()
    return nc


def get_nc(loop_k=None):
    key = (loop_k, tuple(sorted(CONFIG.items())))
    if key not in _NC_CACHE:
        _NC_CACHE[key] = _build_nc(L=L, loop_k=loop_k, **CONFIG)
    return _NC_CACHE[key]


def make_in_maps(x, push_gate, pop_gate):
    pg = np.ascontiguousarray(push_gate.reshape(B_TOTAL, L))
    og = np.ascontiguousarray(pop_gate.reshape(B_TOTAL, L))
    maps = [
        {
            "x": x[c * B_LOC : (c + 1) * B_LOC],
            "pg": pg[c * B_LOC : (c + 1) * B_LOC],
            "og": og[c * B_LOC : (c + 1) * B_LOC],
        }
        for c in range(N_CORES)
    ]
    if not CONFIG["gpsimd_identity"]:
        eye = np.eye(128, dtype=np.float32)
        for m in maps:
            m["ident"] = eye
    return maps


def assemble_out(results):
    # each core's "out" is [D, B_LOC]; full output is [B_TOTAL, D]
    return np.concatenate(
        [np.asarray(results[c]["out"]).T for c in range(N_CORES)], axis=0
    )


def kernel(x, push_gate, pop_gate):
    from concourse.bass_utils import run_bass_kernel_spmd

    x = np.ascontiguousarray(np.asarray(x, dtype=np.float32))
    nc = get_nc()
    in_maps = make_in_maps(
        x,
        np.asarray(push_gate, dtype=np.float32),
        np.asarray(pop_gate, dtype=np.float32),
    )
    res = run_bass_kernel_spmd(nc, in_maps, list(range(N_CORES)))
    return assemble_out(res.results).astype(np.float32)
